# revision 17
# baseline (speedup 1.0000x reference)
# Trainium2 Bass kernel for ClassAttn (single class-token query attention).
#
# Math (per batch b):
#   q   = x[b,0] @ Wq * scale                       [CR]
#   logits[h,n] = sum_d q[h,d] * (x[b] @ Wk)[n,h,d]
#               = sum_c x[b,n,c] * wq_eff[c,h]      with wq_eff[c,h] = sum_d Wk[c,h*HD+d] q[h*HD+d]
#   w = exp(logits)          (inputs are bounded; softmax needs no max-subtraction)
#   z[h] = sum_n w[h,n]
#   s[h,c] = sum_n w[h,n] x[n,c]                    (attn-weighted token sum)
#   o[h,d] = (1/z[h]) sum_c s[h,c] Wv[c,h*HD+d]
#   out = o.flatten() @ Wp + bp
#
# This avoids materializing K and V entirely: the heavy work is two
# [N,C]-sized streaming contractions (logits and s) per batch instead of two
# [N,C]x[C,CR] projections — 16x fewer FLOPs.
#
# Sharding: data-parallel over batch. 8 cores x 4 batches each; weights
# replicated; no collectives. Per-core x shard is streamed in groups of 512
# tokens, cast fp32->bf16 in the DMA (SWDGE cast), transposed on the PE
# (the logits contraction is over c, which must live on partitions), and
# consumed twice (logits from x^T, s-accum from x natural).
#
# v4 engine-balance notes:
#  - C1 (logits, M=16 output) is 2x column-tiled: the two c-block-halves
#    stream concurrently through disjoint 32-column strips of the PE array.
#    The partials are NEVER added: exp factorizes (exp(a+b)=exp(a)exp(b)),
#    so two ACT exps read the two PSUM strips directly and one DVE
#    scalar_tensor_tensor multiplies them into w while accumulating z.
#  - C2 (s-accum, M=16) is 4x column-tiled over token-blocks; the four
#    partial s strips (PSUM partitions 32*blk) are folded once per batch by
#    a fold-mask matmul, not per-slot DVE adds.
#  - x^T tiles transpose in transpose-mode (bf16 PSUM halves the PSUM->SBUF
#    copy cost); copies alternate DVE/ACT.

import numpy as np
from contextlib import ExitStack

import concourse.bass as bass
import concourse.mybir as mybir
import concourse.tile as tile
from concourse import bacc
from concourse.masks import make_identity

F32 = mybir.dt.float32
BF16 = mybir.dt.bfloat16

B, N, C = 32, 4096, 1024
H, HD = 16, 16
CR = 256
SCALE = HD ** -0.5
NCORES = 8
BS = B // NCORES          # batches per core
GTOK = 512                # tokens per group
BLK = 128                 # tokens per block (partition tile)
NBLK = GTOK // BLK        # 4 blocks per group
NCB = C // 128            # 8 c-blocks


def emit(tc, x_d, wq_d, wk_d, wv_d, wp_d, bp_d, dmask_d, fmask_d, out_d, bs, n):
    nc = tc.nc
    ngroups = n // GTOK
    with ExitStack() as ctx:
        const = ctx.enter_context(tc.tile_pool(name="const", bufs=1))
        px = ctx.enter_context(tc.tile_pool(name="px", bufs=8))
        pxt = ctx.enter_context(tc.tile_pool(name="pxt", bufs=4))
        pw = ctx.enter_context(tc.tile_pool(name="pw", bufs=6))
        pb = ctx.enter_context(tc.tile_pool(name="pb", bufs=2))
        ps_xt = ctx.enter_context(tc.tile_pool(name="ps_xt", bufs=2, space="PSUM"))
        ps_lg = ctx.enter_context(tc.tile_pool(name="ps_lg", bufs=2, space="PSUM"))
        ps_w = ctx.enter_context(tc.tile_pool(name="ps_w", bufs=1, space="PSUM"))
        ps_sm = ctx.enter_context(tc.tile_pool(name="ps_sm", bufs=1, space="PSUM"))
        ps_s = ctx.enter_context(tc.tile_pool(name="ps_s", bufs=1, space="PSUM"))

        # ---- constants / weights ----
        ident = const.tile([128, 128], BF16)
        make_identity(nc, ident[:])
        ones_row = const.tile([1, 128], BF16)
        nc.vector.memset(ones_row[:], 1.0)

        # All weights ride the sync (HWDGE) queue so the SWDGE queue carries
        # only the x stream: fp32 loads + on-chip casts for the bf16 copies.
        ptmp = ctx.enter_context(tc.tile_pool(name="ptmp", bufs=1))
        wq_sb = const.tile([128, NCB, CR], BF16)     # Wq[c,r] c-blocked, bf16
        wq_f = ptmp.tile([128, NCB, CR], F32, tag="wtmpA")
        nc.sync.dma_start(out=wq_f[:], in_=wq_d.rearrange("(j p) r -> p j r", p=128))
        nc.scalar.copy(wq_sb[:], wq_f[:])
        wk_sb = const.tile([128, NCB, CR], F32)      # Wk[c,r] fp32 (for wq_eff)
        nc.sync.dma_start(out=wk_sb[:], in_=wk_d.rearrange("(j p) r -> p j r", p=128))
        bp_sb = const.tile([1, C], F32)
        nc.sync.dma_start(out=bp_sb[:], in_=bp_d.rearrange("(u c) -> u c", u=1))
        dmask_sb = const.tile([128, 2, H], F32)   # dmask[p,half,h] = (h == 8*half + p//16)
        nc.sync.dma_start(out=dmask_sb[:], in_=dmask_d)
        wv_sb = const.tile([128, NCB, CR], BF16)     # Wv[c,r]
        wp_sb = const.tile([128, 2, C], BF16)        # Wp[r,c] r-blocked
        fmask_sb = const.tile([128, H], BF16)     # fmask[p,h] = (p%32==h)&&(p%32<16)

        def emit_deferred_weights():
            wv_f = ptmp.tile([128, NCB, CR], F32, tag="wtmpA")
            nc.sync.dma_start(
                out=wv_f[:], in_=wv_d.rearrange("(j p) r -> p j r", p=128))
            nc.vector.tensor_copy(wv_sb[:], wv_f[:])
            wp_f = ptmp.tile([128, 2, C], F32, tag="wtmpB")
            nc.sync.dma_start(
                out=wp_f[:], in_=wp_d.rearrange("(j p) c -> p j c", p=128))
            nc.scalar.copy(wp_sb[:], wp_f[:])
            nc.gpsimd.dma_start(out=fmask_sb[:], in_=fmask_d)

        bstate = {}   # per-batch: s_ps, zg_all, wq_eff_bf
        gstate = {}   # per-(b,g): xg, xt
        wstate = {}   # per-(b,g): wT (exp-product output, consumed by C2)

        def emit_T(b, g):
            """Load + cast one 512-token group, transpose to xT via PE."""
            xg = px.tile([128, NBLK, C], BF16, tag="xg")
            # Token n' = t*128+p of this group holds DRAM token g*GTOK + 4p + t:
            # each partition reads 4 consecutive rows = 16 KB contiguous DRAM
            # per descriptor (4x fewer descriptors; attention is permutation-
            # invariant over tokens so any consistent order works).
            nc.gpsimd.dma_start(
                out=xg[:],
                in_=x_d[b, g * GTOK : (g + 1) * GTOK, :].rearrange(
                    "(p t) c -> p t c", t=NBLK
                ),
            )
            xt = pxt.tile([128, NCB, GTOK], BF16, tag="xt")
            for blk in range(NBLK):
                for jh in range(2):          # two half-bank psum tiles per block
                    xt_ps = ps_xt.tile([128, 4, 128], BF16, tag="xt_ps")
                    for jj in range(4):
                        j = jh * 4 + jj
                        nc.tensor.transpose(
                            xt_ps[:, jj, :],
                            xg[:, blk, j * 128 : (j + 1) * 128], ident[:],
                        )
                    dst = xt[:, jh * 4 : (jh + 1) * 4, blk * BLK : (blk + 1) * BLK]
                    if jh == 0:
                        nc.vector.tensor_copy(dst, xt_ps[:])
                    else:
                        nc.scalar.copy(dst, xt_ps[:])
            gstate[(b, g)] = (xg, xt)

        def emit_PRO(b):
            """q from xT of group 0, then wq_eff (DVE chain overlaps next slots)."""
            _, xt0 = gstate[(b, 0)]
            s_ps = ps_s.tile([128, C], F32, tag="s")
            zg_all = pb.tile([16, ngroups], F32, tag="zg")
            q_ps = ps_sm.tile([1, CR], F32, tag="sm")
            for j in range(NCB):
                nc.tensor.matmul(
                    q_ps[:], xt0[:, j, 0:1], wq_sb[:, j, :],
                    start=(j == 0), stop=(j == NCB - 1),
                )
            qs_bf = pb.tile([1, CR], BF16, tag="qs")
            nc.scalar.mul(qs_bf[:], q_ps[:], SCALE)
            rep_ps = ps_sm.tile([128, CR], F32, tag="sm")
            nc.tensor.matmul(rep_ps[:], ones_row[:], qs_bf[:])
            qs_rep = pb.tile([128, CR], F32, tag="qs_rep")
            nc.vector.tensor_copy(qs_rep[:], rep_ps[:])
            wq_eff = pb.tile([128, NCB, H], F32, tag="wq_eff")
            tmp = pb.tile([128, CR], F32, tag="tmp")
            for j in range(NCB):
                nc.vector.tensor_mul(tmp[:], wk_sb[:, j, :], qs_rep[:])
                nc.vector.reduce_sum(
                    wq_eff[:, j, :],
                    tmp.rearrange("p (h d) -> p h d", h=H),
                    axis=mybir.AxisListType.X,
                )
            wq_eff_bf = pb.tile([128, NCB, H], BF16, tag="wq_eff_bf")
            nc.vector.tensor_copy(wq_eff_bf[:], wq_eff[:])
            bstate[b] = (s_ps, zg_all, wq_eff_bf)

        def emit_C1(b, g):
            """logits 2x col-tiled; exp of the two strips multiplied (+z)."""
            _, xt = gstate[(b, g)]
            s_ps, zg_all, wq_eff_bf = bstate[b]
            lg_ps = ps_lg.tile([128, GTOK], F32, tag="lg")
            for r in range(4):
                nc.tensor.matmul(
                    lg_ps[0:16, :], wq_eff_bf[:, r, :], xt[:, r, :],
                    start=(r == 0), stop=(r == 3),
                )
                nc.tensor.matmul(
                    lg_ps[32:48, :], wq_eff_bf[:, 4 + r, :], xt[:, 4 + r, :],
                    start=(r == 0), stop=(r == 3),
                )
            e0 = pw.tile([16, GTOK], BF16, tag="e0")
            nc.scalar.activation(e0[:], lg_ps[0:16, :],
                                 mybir.ActivationFunctionType.Exp)
            e1 = pw.tile([16, GTOK], BF16, tag="e1")
            nc.scalar.activation(e1[:], lg_ps[32:48, :],
                                 mybir.ActivationFunctionType.Exp)
            wT = pw.tile([16, GTOK], BF16, tag="wT")
            nc.vector.scalar_tensor_tensor(
                wT[:], e0[:], 1.0, e1[:],
                op0=mybir.AluOpType.mult, op1=mybir.AluOpType.mult,
                accum_out=zg_all[:, g : g + 1],
            )
            wstate[(b, g)] = wT

        def emit_C2(b, g):
            """w transposes (PE matmuls), then column-tiled s accumulation."""
            xg, _ = gstate.pop((b, g))
            wT = wstate.pop((b, g))
            s_ps, zg_all, _ = bstate[b]
            w_ps = ps_w.tile([128, NBLK, H], F32, tag="w")
            for blk in range(NBLK):
                nc.tensor.matmul(
                    w_ps[:, blk, :],
                    wT[:, blk * BLK : (blk + 1) * BLK], ident[0:16, 0:16],
                )
            w_sb = pw.tile([128, NBLK, H], BF16, tag="w_sb")
            nc.vector.tensor_copy(w_sb[:], w_ps[:])
            first = g == 0
            last = g == ngroups - 1
            for half in range(2):
                for blk in range(NBLK):
                    nc.tensor.matmul(
                        s_ps[32 * blk : 32 * blk + 16,
                             half * 512 : (half + 1) * 512],
                        w_sb[:, blk, :],
                        xg[:, blk, half * 512 : (half + 1) * 512],
                        start=first, stop=last,
                        tile_position=(0, 32 * blk),
                    )

        def emit_E(b):
            """Fold col-tiled s via mask matmul, normalize, project Wv, Wp+bias."""
            s_ps, zg_all, _ = bstate.pop(b)
            z_tot = pb.tile([16, 1], F32, tag="z_tot")
            nc.vector.reduce_sum(z_tot[:], zg_all[:], axis=mybir.AxisListType.X)
            rz = pb.tile([16, 1], F32, tag="rz")
            nc.vector.reciprocal(rz[:], z_tot[:])
            s_sb = pb.tile([128, C], BF16, tag="s_sb")
            nc.vector.tensor_copy(s_sb[:], s_ps[:])
            sbar = pb.tile([16, C], BF16, tag="sbar")
            for half in range(2):
                sf_ps = ps_lg.tile([128, GTOK], F32, tag="lg")
                nc.tensor.matmul(
                    sf_ps[0:16, :], fmask_sb[:],
                    s_sb[:, half * 512 : (half + 1) * 512],
                )
                nc.vector.tensor_scalar_mul(
                    sbar[:, half * 512 : (half + 1) * 512], sf_ps[0:16, :], rz[:]
                )
            stT = pb.tile([128, NCB, H], BF16, tag="stT")
            st_pss = []
            for j in range(NCB):
                st_ps = ps_sm.tile([128, H], F32, tag="sm")
                nc.tensor.matmul(
                    st_ps[:], sbar[:, j * 128 : (j + 1) * 128], ident[0:16, 0:16],
                )
                st_pss.append(st_ps)
            for j in range(NCB):
                nc.vector.tensor_copy(stT[:, j, :], st_pss[j][:])
            # o_fullT[cr, h] = sum_c Wv[c, cr] * sbar[h, c]; keep only h == cr//HD
            o_flatT_f = pb.tile([128, 2], F32, tag="o_flatT_f")
            o_flatT = pb.tile([128, 2], BF16, tag="o_flatT")
            for half in range(2):
                of_ps = ps_sm.tile([128, H], F32, tag="sm")
                for j in range(NCB):
                    nc.tensor.matmul(
                        of_ps[:], wv_sb[:, j, half * 128 : (half + 1) * 128],
                        stT[:, j, :],
                        start=(j == 0), stop=(j == NCB - 1),
                    )
                om = pb.tile([128, H], F32, tag="om")
                nc.vector.tensor_mul(om[:], of_ps[:], dmask_sb[:, half, :])
                nc.vector.reduce_sum(
                    o_flatT_f[:, half : half + 1], om[:], axis=mybir.AxisListType.X
                )
            nc.vector.tensor_copy(o_flatT[:], o_flatT_f[:])
            # out = o_flat @ Wp + bp
            out_sb = pb.tile([1, C], F32, tag="out_sb")
            for half in range(2):
                op_ps = ps_sm.tile([1, 512], F32, tag="sm")
                for j in range(2):
                    nc.tensor.matmul(
                        op_ps[:], o_flatT[:, j : j + 1],
                        wp_sb[:, j, half * 512 : (half + 1) * 512],
                        start=(j == 0), stop=(j == 1),
                    )
                nc.vector.tensor_add(
                    out_sb[:, half * 512 : (half + 1) * 512], op_ps[:],
                    bp_sb[:, half * 512 : (half + 1) * 512],
                )
            nc.sync.dma_start(out=out_d[b : b + 1, :], in_=out_sb[:])

        # ---- software-pipelined emission ----
        # slot k: T(k) [+PRO at batch start], E (2 slots after last C2 of a
        # batch), C2(k-2), C1(k-1). PE never waits on a same-slot cross-engine
        # round-trip; exp/drains have a full slot of PE work to hide under.
        from collections import deque
        items = [(b, g) for b in range(bs) for g in range(ngroups)]
        q_c1 = deque()   # waiting to be logits'd (2-slot lag)
        q_c2 = deque()   # waiting for w+s (1 more slot after C1)
        pend_epi = None
        def flush_slot():
            nonlocal pend_epi
            if pend_epi is not None:
                emit_E(pend_epi)
                pend_epi = None
            if q_c2:
                it2 = q_c2.popleft()
                emit_C2(*it2)
                if it2[1] == ngroups - 1:
                    pend_epi = it2[0]
            if len(q_c1) >= 2:
                it1 = q_c1.popleft()
                emit_C1(*it1)
                q_c2.append(it1)
        for idx, it in enumerate(items):
            emit_T(*it)
            if it[1] == 0:
                emit_PRO(it[0])
            if idx == 1:
                emit_deferred_weights()
            flush_slot()
            q_c1.append(it)
        for _ in range(5):
            if len(q_c1) == 1:
                emit_C1(*q_c1[0])
                q_c2.append(q_c1.popleft())
            flush_slot()


def make_dmask():
    dm = np.zeros((128, 2, H), dtype=np.float32)
    for p in range(128):
        for half in range(2):
            dm[p, half, 8 * half + p // 16] = 1.0
    return dm


def make_fmask():
    fm = np.zeros((128, H), dtype=np.float32)
    for p in range(128):
        if p % 32 < 16:
            fm[p, p % 32] = 1.0
    return fm


def build_bass(bs=BS, n=N):
    nc = bacc.Bacc("TRN2", target_bir_lowering=False, debug=False, num_devices=NCORES)
    x_d = nc.dram_tensor("x", [bs, n, C], F32, kind="ExternalInput").ap()
    wq_d = nc.dram_tensor("Wq", [C, CR], F32, kind="ExternalInput").ap()
    wk_d = nc.dram_tensor("Wk", [C, CR], F32, kind="ExternalInput").ap()
    wv_d = nc.dram_tensor("Wv", [C, CR], F32, kind="ExternalInput").ap()
    wp_d = nc.dram_tensor("Wp", [CR, C], F32, kind="ExternalInput").ap()
    bp_d = nc.dram_tensor("bp", [C], F32, kind="ExternalInput").ap()
    dmask_d = nc.dram_tensor("dmask", [128, 2, H], F32, kind="ExternalInput").ap()
    fmask_d = nc.dram_tensor("fmask", [128, H], F32, kind="ExternalInput").ap()
    out_d = nc.dram_tensor("out", [bs, C], F32, kind="ExternalOutput").ap()
    with tile.TileContext(nc) as tc:
        emit(tc, x_d, wq_d, wk_d, wv_d, wp_d, bp_d, dmask_d, fmask_d, out_d, bs, n)
    nc.compile()
    return nc


def kernel(**inputs):
    from concourse.bass_utils import run_bass_kernel_spmd

    x = np.ascontiguousarray(np.asarray(inputs["x"], dtype=np.float32))
    wq = np.ascontiguousarray(np.asarray(inputs["Wq"], dtype=np.float32))
    wk = np.ascontiguousarray(np.asarray(inputs["Wk"], dtype=np.float32))
    wv = np.ascontiguousarray(np.asarray(inputs["Wv"], dtype=np.float32))
    wp = np.ascontiguousarray(np.asarray(inputs["Wp"], dtype=np.float32))
    bp = np.ascontiguousarray(np.asarray(inputs["bp"], dtype=np.float32))

    nc = build_bass()
    in_maps = [
        {
            "x": x[c * BS : (c + 1) * BS],
            "Wq": wq, "Wk": wk, "Wv": wv, "Wp": wp, "bp": bp,
            "dmask": make_dmask(), "fmask": make_fmask(),
        }
        for c in range(NCORES)
    ]
    res = run_bass_kernel_spmd(nc, in_maps, core_ids=list(range(NCORES)))
    out = np.concatenate([r["out"] for r in res.results], axis=0)  # [B, C]
    return out.reshape(B, 1, C).astype(np.float32)


# revision 19
# speedup vs baseline: 1.1107x; 1.1107x over previous
# Trainium2 Bass kernel for ClassAttn (single class-token query attention).
#
# Math (per batch b):
#   q   = x[b,0] @ Wq * scale                       [CR]
#   logits[h,n] = sum_d q[h,d] * (x[b] @ Wk)[n,h,d]
#               = sum_c x[b,n,c] * wq_eff[c,h]      with wq_eff[c,h] = sum_d Wk[c,h*HD+d] q[h*HD+d]
#   w = exp(logits)          (inputs are bounded; softmax needs no max-subtraction)
#   z[h] = sum_n w[h,n]
#   s[h,c] = sum_n w[h,n] x[n,c]                    (attn-weighted token sum)
#   o[h,d] = (1/z[h]) sum_c s[h,c] Wv[c,h*HD+d]
#   out = o.flatten() @ Wp + bp
#
# This avoids materializing K and V entirely: the heavy work is two
# [N,C]-sized streaming contractions (logits and s) per batch instead of two
# [N,C]x[C,CR] projections — 16x fewer FLOPs.
#
# Sharding: data-parallel over batch. 8 cores x 4 batches each; weights
# replicated; no collectives. Per-core x shard is streamed in groups of 512
# tokens, cast fp32->bf16 in the DMA (SWDGE cast), transposed on the PE
# (the logits contraction is over c, which must live on partitions), and
# consumed twice (logits from x^T, s-accum from x natural).
#
# v4 engine-balance notes:
#  - C1 (logits, M=16 output) is 2x column-tiled: the two c-block-halves
#    stream concurrently through disjoint 32-column strips of the PE array.
#    The partials are NEVER added: exp factorizes (exp(a+b)=exp(a)exp(b)),
#    so two ACT exps read the two PSUM strips directly and one DVE
#    scalar_tensor_tensor multiplies them into w while accumulating z.
#  - C2 (s-accum, M=16) is 4x column-tiled over token-blocks; the four
#    partial s strips (PSUM partitions 32*blk) are folded once per batch by
#    a fold-mask matmul, not per-slot DVE adds.
#  - x^T tiles transpose in transpose-mode (bf16 PSUM halves the PSUM->SBUF
#    copy cost); copies alternate DVE/ACT.

import numpy as np
from contextlib import ExitStack

import concourse.bass as bass
import concourse.mybir as mybir
import concourse.tile as tile
from concourse import bacc
from concourse.masks import make_identity

F32 = mybir.dt.float32
BF16 = mybir.dt.bfloat16

B, N, C = 32, 4096, 1024
H, HD = 16, 16
CR = 256
SCALE = HD ** -0.5
NCORES = 8
BS = B // NCORES          # batches per core
GTOK = 512                # tokens per group
BLK = 128                 # tokens per block (partition tile)
NBLK = GTOK // BLK        # 4 blocks per group
NCB = C // 128            # 8 c-blocks


def emit(tc, x_d, wq_d, wk_d, wv_d, wp_d, bp_d, dmask_d, fmask_d, out_d, bs, n):
    nc = tc.nc
    ngroups = n // GTOK
    with ExitStack() as ctx:
        const = ctx.enter_context(tc.tile_pool(name="const", bufs=1))
        px = ctx.enter_context(tc.tile_pool(name="px", bufs=8))
        pxt = ctx.enter_context(tc.tile_pool(name="pxt", bufs=4))
        pw = ctx.enter_context(tc.tile_pool(name="pw", bufs=6))
        pb = ctx.enter_context(tc.tile_pool(name="pb", bufs=2))
        ps_xt = ctx.enter_context(tc.tile_pool(name="ps_xt", bufs=2, space="PSUM"))
        ps_lg = ctx.enter_context(tc.tile_pool(name="ps_lg", bufs=2, space="PSUM"))
        ps_w = ctx.enter_context(tc.tile_pool(name="ps_w", bufs=1, space="PSUM"))
        ps_sm = ctx.enter_context(tc.tile_pool(name="ps_sm", bufs=1, space="PSUM"))
        ps_s = ctx.enter_context(tc.tile_pool(name="ps_s", bufs=1, space="PSUM"))

        # ---- constants / weights ----
        ident = const.tile([128, 128], BF16)
        make_identity(nc, ident[:])
        ones_row = const.tile([1, 128], BF16)
        nc.vector.memset(ones_row[:], 1.0)

        # All weights ride the SWDGE queue with the x stream — HWDGE (sync)
        # transfers starve while the SWDGE queue is saturated (weighted QoS
        # is broken on trn2; SWDGE effectively has priority). Small fp32
        # loads (wk/bp/dmask) go on sync early, before the x stream ramps.
        wk_sb = const.tile([128, NCB, CR], F32)      # Wk[c,r] fp32 (for wq_eff)
        nc.sync.dma_start(out=wk_sb[:], in_=wk_d.rearrange("(j p) r -> p j r", p=128))
        bp_sb = const.tile([1, C], F32)
        nc.sync.dma_start(out=bp_sb[:], in_=bp_d.rearrange("(u c) -> u c", u=1))
        dmask_sb = const.tile([128, 2, H], F32)   # dmask[p,half,h] = (h == 8*half + p//16)
        nc.sync.dma_start(out=dmask_sb[:], in_=dmask_d)
        wq_sb = const.tile([128, NCB, CR], BF16)     # Wq[c,r] c-blocked, bf16
        wv_sb = const.tile([128, NCB, CR], BF16)     # Wv[c,r]
        wp_sb = const.tile([128, 2, C], BF16)        # Wp[r,c] r-blocked
        fmask_sb = const.tile([128, H], BF16)     # fmask[p,h] = (p%32==h)&&(p%32<16)

        def emit_wq_weight():
            # emitted right after the first x-group dma so xg0 leads the queue
            nc.gpsimd.dma_start(
                out=wq_sb[:], in_=wq_d.rearrange("(j p) r -> p j r", p=128))

        def emit_deferred_weights():
            nc.gpsimd.dma_start(
                out=wv_sb[:], in_=wv_d.rearrange("(j p) r -> p j r", p=128))
            nc.gpsimd.dma_start(
                out=wp_sb[:], in_=wp_d.rearrange("(j p) c -> p j c", p=128))
            nc.gpsimd.dma_start(out=fmask_sb[:], in_=fmask_d)

        bstate = {}   # per-batch: s_ps, zg_all, wq_eff_bf
        gstate = {}   # per-(b,g): xg, xt
        wstate = {}   # per-(b,g): wT (exp-product output, consumed by C2)

        def emit_T(b, g):
            """Load + cast one 512-token group, transpose to xT via PE."""
            xg = px.tile([128, NBLK, C], BF16, tag="xg")
            # Token n' = t*128+p of this group holds DRAM token g*GTOK + 4p + t:
            # each partition reads 4 consecutive rows = 16 KB contiguous DRAM
            # per descriptor (4x fewer descriptors; attention is permutation-
            # invariant over tokens so any consistent order works).
            nc.gpsimd.dma_start(
                out=xg[:],
                in_=x_d[b, g * GTOK : (g + 1) * GTOK, :].rearrange(
                    "(p t) c -> p t c", t=NBLK
                ),
            )
            xt = pxt.tile([128, NCB, GTOK], BF16, tag="xt")
            for blk in range(NBLK):
                for jh in range(2):          # two half-bank psum tiles per block
                    xt_ps = ps_xt.tile([128, 4, 128], BF16, tag="xt_ps")
                    for jj in range(4):
                        j = jh * 4 + jj
                        nc.tensor.transpose(
                            xt_ps[:, jj, :],
                            xg[:, blk, j * 128 : (j + 1) * 128], ident[:],
                        )
                    dst = xt[:, jh * 4 : (jh + 1) * 4, blk * BLK : (blk + 1) * BLK]
                    if jh == 0:
                        nc.vector.tensor_copy(dst, xt_ps[:])
                    else:
                        nc.scalar.copy(dst, xt_ps[:])
            gstate[(b, g)] = (xg, xt)

        def emit_PRO(b):
            """q from xT of group 0, then wq_eff (DVE chain overlaps next slots)."""
            _, xt0 = gstate[(b, 0)]
            s_ps = ps_s.tile([128, C], F32, tag="s")
            zg_all = pb.tile([16, ngroups], F32, tag="zg")
            q_ps = ps_sm.tile([1, CR], F32, tag="sm")
            for j in range(NCB):
                nc.tensor.matmul(
                    q_ps[:], xt0[:, j, 0:1], wq_sb[:, j, :],
                    start=(j == 0), stop=(j == NCB - 1),
                )
            qs_bf = pb.tile([1, CR], BF16, tag="qs")
            nc.scalar.mul(qs_bf[:], q_ps[:], SCALE)
            rep_ps = ps_sm.tile([128, CR], F32, tag="sm")
            nc.tensor.matmul(rep_ps[:], ones_row[:], qs_bf[:])
            qs_rep = pb.tile([128, CR], F32, tag="qs_rep")
            nc.vector.tensor_copy(qs_rep[:], rep_ps[:])
            wq_eff = pb.tile([128, NCB, H], F32, tag="wq_eff")
            tmp = pb.tile([128, CR], F32, tag="tmp")
            for j in range(NCB):
                nc.vector.tensor_mul(tmp[:], wk_sb[:, j, :], qs_rep[:])
                nc.vector.reduce_sum(
                    wq_eff[:, j, :],
                    tmp.rearrange("p (h d) -> p h d", h=H),
                    axis=mybir.AxisListType.X,
                )
            wq_eff_bf = pb.tile([128, NCB, H], BF16, tag="wq_eff_bf")
            nc.vector.tensor_copy(wq_eff_bf[:], wq_eff[:])
            bstate[b] = (s_ps, zg_all, wq_eff_bf)

        def emit_C1(b, g):
            """logits 2x col-tiled; exp of the two strips multiplied (+z)."""
            _, xt = gstate[(b, g)]
            s_ps, zg_all, wq_eff_bf = bstate[b]
            lg_ps = ps_lg.tile([128, GTOK], F32, tag="lg")
            for r in range(4):
                nc.tensor.matmul(
                    lg_ps[0:16, :], wq_eff_bf[:, r, :], xt[:, r, :],
                    start=(r == 0), stop=(r == 3),
                )
                nc.tensor.matmul(
                    lg_ps[32:48, :], wq_eff_bf[:, 4 + r, :], xt[:, 4 + r, :],
                    start=(r == 0), stop=(r == 3),
                )
            e0 = pw.tile([16, GTOK], BF16, tag="e0")
            nc.scalar.activation(e0[:], lg_ps[0:16, :],
                                 mybir.ActivationFunctionType.Exp)
            e1 = pw.tile([16, GTOK], BF16, tag="e1")
            nc.scalar.activation(e1[:], lg_ps[32:48, :],
                                 mybir.ActivationFunctionType.Exp)
            wT = pw.tile([16, GTOK], BF16, tag="wT")
            nc.vector.scalar_tensor_tensor(
                wT[:], e0[:], 1.0, e1[:],
                op0=mybir.AluOpType.mult, op1=mybir.AluOpType.mult,
                accum_out=zg_all[:, g : g + 1],
            )
            wstate[(b, g)] = wT

        def emit_C2(b, g):
            """w transposes (PE matmuls), then column-tiled s accumulation."""
            xg, _ = gstate.pop((b, g))
            wT = wstate.pop((b, g))
            s_ps, zg_all, _ = bstate[b]
            w_ps = ps_w.tile([128, NBLK, H], F32, tag="w")
            for blk in range(NBLK):
                nc.tensor.matmul(
                    w_ps[:, blk, :],
                    wT[:, blk * BLK : (blk + 1) * BLK], ident[0:16, 0:16],
                )
            w_sb = pw.tile([128, NBLK, H], BF16, tag="w_sb")
            nc.vector.tensor_copy(w_sb[:], w_ps[:])
            first = g == 0
            last = g == ngroups - 1
            for half in range(2):
                for blk in range(NBLK):
                    nc.tensor.matmul(
                        s_ps[32 * blk : 32 * blk + 16,
                             half * 512 : (half + 1) * 512],
                        w_sb[:, blk, :],
                        xg[:, blk, half * 512 : (half + 1) * 512],
                        start=first, stop=last,
                        tile_position=(0, 32 * blk),
                    )

        def emit_E(b):
            """Fold col-tiled s via mask matmul, normalize, project Wv, Wp+bias."""
            s_ps, zg_all, _ = bstate.pop(b)
            z_tot = pb.tile([16, 1], F32, tag="z_tot")
            nc.vector.reduce_sum(z_tot[:], zg_all[:], axis=mybir.AxisListType.X)
            rz = pb.tile([16, 1], F32, tag="rz")
            nc.vector.reciprocal(rz[:], z_tot[:])
            s_sb = pb.tile([128, C], BF16, tag="s_sb")
            nc.vector.tensor_copy(s_sb[:], s_ps[:])
            sbar = pb.tile([16, C], BF16, tag="sbar")
            for half in range(2):
                sf_ps = ps_lg.tile([128, GTOK], F32, tag="lg")
                nc.tensor.matmul(
                    sf_ps[0:16, :], fmask_sb[:],
                    s_sb[:, half * 512 : (half + 1) * 512],
                )
                nc.vector.tensor_scalar_mul(
                    sbar[:, half * 512 : (half + 1) * 512], sf_ps[0:16, :], rz[:]
                )
            stT = pb.tile([128, NCB, H], BF16, tag="stT")
            st_pss = []
            for j in range(NCB):
                st_ps = ps_sm.tile([128, H], F32, tag="sm")
                nc.tensor.matmul(
                    st_ps[:], sbar[:, j * 128 : (j + 1) * 128], ident[0:16, 0:16],
                )
                st_pss.append(st_ps)
            for j in range(NCB):
                nc.vector.tensor_copy(stT[:, j, :], st_pss[j][:])
            # o_fullT[cr, h] = sum_c Wv[c, cr] * sbar[h, c]; keep only h == cr//HD
            o_flatT_f = pb.tile([128, 2], F32, tag="o_flatT_f")
            o_flatT = pb.tile([128, 2], BF16, tag="o_flatT")
            for half in range(2):
                of_ps = ps_sm.tile([128, H], F32, tag="sm")
                for j in range(NCB):
                    nc.tensor.matmul(
                        of_ps[:], wv_sb[:, j, half * 128 : (half + 1) * 128],
                        stT[:, j, :],
                        start=(j == 0), stop=(j == NCB - 1),
                    )
                om = pb.tile([128, H], F32, tag="om")
                nc.vector.tensor_mul(om[:], of_ps[:], dmask_sb[:, half, :])
                nc.vector.reduce_sum(
                    o_flatT_f[:, half : half + 1], om[:], axis=mybir.AxisListType.X
                )
            nc.vector.tensor_copy(o_flatT[:], o_flatT_f[:])
            # out = o_flat @ Wp + bp
            out_sb = pb.tile([1, C], F32, tag="out_sb")
            for half in range(2):
                op_ps = ps_sm.tile([1, 512], F32, tag="sm")
                for j in range(2):
                    nc.tensor.matmul(
                        op_ps[:], o_flatT[:, j : j + 1],
                        wp_sb[:, j, half * 512 : (half + 1) * 512],
                        start=(j == 0), stop=(j == 1),
                    )
                nc.vector.tensor_add(
                    out_sb[:, half * 512 : (half + 1) * 512], op_ps[:],
                    bp_sb[:, half * 512 : (half + 1) * 512],
                )
            nc.sync.dma_start(out=out_d[b : b + 1, :], in_=out_sb[:])

        # ---- software-pipelined emission ----
        # slot k: T(k) [+PRO at batch start], E (2 slots after last C2 of a
        # batch), C2(k-2), C1(k-1). PE never waits on a same-slot cross-engine
        # round-trip; exp/drains have a full slot of PE work to hide under.
        from collections import deque
        items = [(b, g) for b in range(bs) for g in range(ngroups)]
        q_c1 = deque()   # waiting to be logits'd (2-slot lag)
        q_c2 = deque()   # waiting for w+s (1 more slot after C1)
        pend_epi = None
        def flush_slot():
            nonlocal pend_epi
            if pend_epi is not None:
                emit_E(pend_epi)
                pend_epi = None
            if q_c2:
                it2 = q_c2.popleft()
                emit_C2(*it2)
                if it2[1] == ngroups - 1:
                    pend_epi = it2[0]
            if len(q_c1) >= 2:
                it1 = q_c1.popleft()
                emit_C1(*it1)
                q_c2.append(it1)
        for idx, it in enumerate(items):
            emit_T(*it)
            if idx == 0:
                emit_wq_weight()
            if it[1] == 0:
                emit_PRO(it[0])
            if idx == 1:
                emit_deferred_weights()
            flush_slot()
            q_c1.append(it)
        for _ in range(5):
            if len(q_c1) == 1:
                emit_C1(*q_c1[0])
                q_c2.append(q_c1.popleft())
            flush_slot()


def make_dmask():
    dm = np.zeros((128, 2, H), dtype=np.float32)
    for p in range(128):
        for half in range(2):
            dm[p, half, 8 * half + p // 16] = 1.0
    return dm


def make_fmask():
    fm = np.zeros((128, H), dtype=np.float32)
    for p in range(128):
        if p % 32 < 16:
            fm[p, p % 32] = 1.0
    return fm


def build_bass(bs=BS, n=N):
    nc = bacc.Bacc("TRN2", target_bir_lowering=False, debug=False, num_devices=NCORES)
    x_d = nc.dram_tensor("x", [bs, n, C], F32, kind="ExternalInput").ap()
    wq_d = nc.dram_tensor("Wq", [C, CR], F32, kind="ExternalInput").ap()
    wk_d = nc.dram_tensor("Wk", [C, CR], F32, kind="ExternalInput").ap()
    wv_d = nc.dram_tensor("Wv", [C, CR], F32, kind="ExternalInput").ap()
    wp_d = nc.dram_tensor("Wp", [CR, C], F32, kind="ExternalInput").ap()
    bp_d = nc.dram_tensor("bp", [C], F32, kind="ExternalInput").ap()
    dmask_d = nc.dram_tensor("dmask", [128, 2, H], F32, kind="ExternalInput").ap()
    fmask_d = nc.dram_tensor("fmask", [128, H], F32, kind="ExternalInput").ap()
    out_d = nc.dram_tensor("out", [bs, C], F32, kind="ExternalOutput").ap()
    with tile.TileContext(nc) as tc:
        emit(tc, x_d, wq_d, wk_d, wv_d, wp_d, bp_d, dmask_d, fmask_d, out_d, bs, n)
    nc.compile()
    return nc


def kernel(**inputs):
    from concourse.bass_utils import run_bass_kernel_spmd

    x = np.ascontiguousarray(np.asarray(inputs["x"], dtype=np.float32))
    wq = np.ascontiguousarray(np.asarray(inputs["Wq"], dtype=np.float32))
    wk = np.ascontiguousarray(np.asarray(inputs["Wk"], dtype=np.float32))
    wv = np.ascontiguousarray(np.asarray(inputs["Wv"], dtype=np.float32))
    wp = np.ascontiguousarray(np.asarray(inputs["Wp"], dtype=np.float32))
    bp = np.ascontiguousarray(np.asarray(inputs["bp"], dtype=np.float32))

    nc = build_bass()
    in_maps = [
        {
            "x": x[c * BS : (c + 1) * BS],
            "Wq": wq, "Wk": wk, "Wv": wv, "Wp": wp, "bp": bp,
            "dmask": make_dmask(), "fmask": make_fmask(),
        }
        for c in range(NCORES)
    ]
    res = run_bass_kernel_spmd(nc, in_maps, core_ids=list(range(NCORES)))
    out = np.concatenate([r["out"] for r in res.results], axis=0)  # [B, C]
    return out.reshape(B, 1, C).astype(np.float32)
